# revision 5
# baseline (speedup 1.0000x reference)
"""Bahdanau-style attention forward on 8 TRN2 NeuronCores.

Per-batch data parallel: batch b -> core b. Each core computes
  S = Q @ K^T          (2048x2048, f32 accuracy via bf16 hi/lo 3-pass split)
  A = softmax(S)       (row softmax with max subtraction)
  C = A @ V            (bf16)
and returns both A (attn) and C (context).

Self-contained: hardcodes shapes B=8, Lq=Lk=2048, D=1024, f32 I/O.
"""

import sys

sys.path.insert(0, "/opt/trn_rl_repo")

import numpy as np

B = 8
LQ = 2048
LK = 2048
D = 1024
P = 128
NQ = LQ // P  # 16 q tiles
NK = LK // P  # 16 kv tiles
ND = D // P  # 8 contraction chunks
NKG = 4  # kv column groups for mm1 rhs (each 512 wide)
KG = LK // NKG  # 512

# set by test harness to enable neuron-profile trace
TRACE = False
LAST_RESULTS = None

_CACHE = {}


def _build_nc():
    import concourse.bass as bass
    import concourse.tile as tile
    from concourse import bacc, mybir

    FP = mybir.dt.float32
    BF = mybir.dt.bfloat16
    AX = mybir.AxisListType
    AF = mybir.ActivationFunctionType

    nc = bacc.Bacc(None, target_bir_lowering=False, debug=False)
    q_in = nc.declare_dram_parameter("q_in", [LQ, D], FP, isOutput=False)
    kv_in = nc.declare_dram_parameter("kv_in", [LK, D], FP, isOutput=False)
    attn_out = nc.declare_dram_parameter("attn", [LQ, LK], FP, isOutput=True)
    ctx_out = nc.declare_dram_parameter("context", [LQ, D], FP, isOutput=True)

    with tile.TileContext(nc) as tc:
        from contextlib import ExitStack

        with ExitStack() as ctx:
            kvload = ctx.enter_context(tc.tile_pool(name="kvload", bufs=3))
            klscr = ctx.enter_context(tc.tile_pool(name="klscr", bufs=3))
            qload = ctx.enter_context(tc.tile_pool(name="qload", bufs=2))
            qsplit = ctx.enter_context(tc.tile_pool(name="qsplit", bufs=2))
            qtp = ctx.enter_context(tc.tile_pool(name="qtp", bufs=2))
            res = ctx.enter_context(tc.tile_pool(name="res", bufs=1))
            epool = ctx.enter_context(tc.tile_pool(name="epool", bufs=2))
            apool = ctx.enter_context(tc.tile_pool(name="apool", bufs=2))
            abfp = ctx.enter_context(tc.tile_pool(name="abfp", bufs=2))
            atp = ctx.enter_context(tc.tile_pool(name="atp", bufs=2))
            cp = ctx.enter_context(tc.tile_pool(name="cp", bufs=2))
            stats = ctx.enter_context(tc.tile_pool(name="stats", bufs=3))
            spsum = ctx.enter_context(tc.tile_pool(name="spsum", bufs=1, space="PSUM"))
            cpsum = ctx.enter_context(tc.tile_pool(name="cpsum", bufs=2, space="PSUM"))

            # Resident: V in bf16 (= Kh), and transposed hi/lo K in 4 column groups
            kh = res.tile([P, NK, D], BF, name="kh", tag="kh")
            khT = [
                res.tile([P, ND, KG], BF, name=f"khT{g}", tag=f"khT{g}")
                for g in range(NKG)
            ]
            klT = [
                res.tile([P, ND, KG], BF, name=f"klT{g}", tag=f"klT{g}")
                for g in range(NKG)
            ]

            # ---- KV prep: load, hi/lo split, transpose ----
            for i in range(NK):
                kvt = kvload.tile([P, D], FP, name="kvt", tag="kvt")
                nc.sync.dma_start(kvt[:], kv_in[i * P : (i + 1) * P, :])
                nc.vector.tensor_copy(kh[:, i, :], kvt[:])
                klt = klscr.tile([P, D], BF, name="klt", tag="klt")
                nc.vector.tensor_sub(klt[:], kvt[:], kh[:, i, :])
                g, c = i // (NK // NKG), i % (NK // NKG)
                nc.sync.dma_start(
                    khT[g][:, :, c * P : (c + 1) * P], kh[:, i, :], transpose=True
                )
                nc.sync.dma_start(
                    klT[g][:, :, c * P : (c + 1) * P], klt[:], transpose=True
                )

            def emit_qchain(i):
                qt = qload.tile([P, D], FP, name="qt", tag="qt")
                nc.sync.dma_start(qt[:], q_in[i * P : (i + 1) * P, :])
                qh = qsplit.tile([P, D], BF, name="qh", tag="qh")
                ql = qsplit.tile([P, D], BF, name="ql", tag="ql")
                nc.vector.tensor_copy(qh[:], qt[:])
                nc.vector.tensor_sub(ql[:], qt[:], qh[:])
                qhT = qtp.tile([P, ND, P], BF, name="qhT", tag="qhT")
                qlT = qtp.tile([P, ND, P], BF, name="qlT", tag="qlT")
                nc.sync.dma_start(qhT[:], qh[:], transpose=True)
                nc.sync.dma_start(qlT[:], ql[:], transpose=True)
                return qhT, qlT

            def emit_mm1(qhT, qlT):
                s = spsum.tile([P, LK], FP, name="spsum_t", tag="spsum_t")
                for g in range(NKG):
                    n = 0
                    total = 3 * ND
                    for j in range(ND):
                        for lhs, rhs in (
                            (qhT, khT[g]),
                            (qhT, klT[g]),
                            (qlT, khT[g]),
                        ):
                            nc.tensor.matmul(
                                s[:, g * KG : (g + 1) * KG],
                                lhsT=lhs[:, j, :],
                                rhs=rhs[:, j, :],
                                start=(n == 0),
                                stop=(n == total - 1),
                            )
                            n += 1
                return s

            def emit_softmax(i, s):
                negmax = stats.tile([P, 1], FP, name="negmax", tag="negmax")
                nc.vector.reduce_max(negmax[:], s[:], axis=AX.X, negate=True)
                e = epool.tile([P, LK], FP, name="e", tag="e")
                zsum = stats.tile([P, 1], FP, name="zsum", tag="zsum")
                nc.scalar.activation(
                    e[:], s[:], AF.Exp, bias=negmax[:], scale=1.0, accum_out=zsum[:]
                )
                inv = stats.tile([P, 1], FP, name="inv", tag="inv")
                nc.vector.reciprocal(inv[:], zsum[:])
                a = apool.tile([P, LK], FP, name="a", tag="a")
                nc.scalar.activation(a[:], e[:], AF.Copy, bias=0.0, scale=inv[:])
                nc.sync.dma_start(attn_out[i * P : (i + 1) * P, :], a[:])
                abf = abfp.tile([P, LK], BF, name="abf", tag="abf")
                nc.vector.tensor_scalar_mul(abf[:], e[:], inv[:])
                at = atp.tile([P, NK, P], BF, name="at", tag="at")
                nc.sync.dma_start(at[:], abf[:], transpose=True)
                return at

            def emit_mm2(at):
                c = cpsum.tile([P, D], FP, name="cpsum_t", tag="cpsum_t")
                ncw = max(1, D // 512)
                w = D // ncw
                for n in range(ncw):
                    for j in range(NK):
                        nc.tensor.matmul(
                            c[:, n * w : (n + 1) * w],
                            lhsT=at[:, j, :],
                            rhs=kh[:, j, n * w : (n + 1) * w],
                            start=(j == 0),
                            stop=(j == NK - 1),
                        )
                return c

            def emit_cout(i, c):
                csb = cp.tile([P, D], FP, name="csb", tag="csb")
                nc.scalar.activation(csb[:], c[:], AF.Copy)
                nc.sync.dma_start(ctx_out[i * P : (i + 1) * P, :], csb[:])

            # ---- software-pipelined main loop ----
            qT = emit_qchain(0)
            s = emit_mm1(*qT)
            cprev = None
            for i in range(NQ):
                if i + 1 < NQ:
                    qT_next = emit_qchain(i + 1)
                at = emit_softmax(i, s)
                if i + 1 < NQ:
                    s = emit_mm1(*qT_next)
                if cprev is not None:
                    emit_cout(i - 1, cprev)
                cprev = emit_mm2(at)
            emit_cout(NQ - 1, cprev)

    nc.finalize()
    return nc


def kernel(output, inputs):
    global LAST_RESULTS
    from concourse.bass_utils import run_bass_kernel_spmd

    if "nc" not in _CACHE:
        _CACHE["nc"] = _build_nc()
    nc = _CACHE["nc"]

    output = np.ascontiguousarray(np.asarray(output, dtype=np.float32))
    inputs = np.ascontiguousarray(np.asarray(inputs, dtype=np.float32))
    in_maps = [{"q_in": output[b], "kv_in": inputs[b]} for b in range(B)]
    r = run_bass_kernel_spmd(nc, in_maps, core_ids=list(range(B)), trace=TRACE)
    LAST_RESULTS = r
    context = np.stack([r.results[b]["context"] for b in range(B)])
    attn = np.stack([r.results[b]["attn"] for b in range(B)])
    return context, attn
